# revision 10
# baseline (speedup 1.0000x reference)
"""BiCut loss kernel for Trainium2, data-parallel over 8 NeuronCores.

Computes sum(output * r) / B where r[i,j] = [0.7, 0] if labels[i,j]==1
else [0, 1.3]  (alpha=0.65, r=0.5).

Strategy: shard batch dim B=8192 across 8 cores (1024 rows each). Each core
streams its 16 MiB output shard + 16 MiB int64 label shard from HBM in
[128 x 4096] tiles and fuses the masked select + reduction into three engine
ops per tile:
  DVE  tensor_tensor_reduce:  sum(0.7 * o0 * m)          (m = label in {0,1})
  DVE  tensor_tensor_reduce:  sum(-1.3 * o1 * m)
  ACT  activation(Copy,accum): sum(1.3 * o1)
so total = sum over slots. int64 labels are viewed host-side as int32 pairs
(little-endian: even words carry the 0/1 value) and only the even words feed
the multiplies (strided AP); the engines convert int32 -> f32 on read.
Per-core partial sums [128, 24] are DMA'd out and reduced on host.
"""

import os
import sys

sys.path.insert(0, "/opt/trn_rl_repo")

import numpy as np

B, L = 8192, 2048
M = 8                      # cores
BC = B // M                # 1024 rows per core
P = 128                    # SBUF partitions
NT = BC // P               # 8 row-tiles per core
ALPHA, R = 0.65, 0.5
W_POS = (1.0 - ALPHA) / R          # 0.7, weight of channel 0 when label==1
W_NEG = ALPHA / (1.0 - R)          # 1.3, weight of channel 1 when label!=1

_NC = {}
LAST = None  # last BassKernelResults, for test harness introspection


def _build(pairs):
    """Build the per-core program. pairs=True when labels arrive as int64
    (viewed as int32 [value, 0] pairs, so the value words are stride-2);
    pairs=False when labels are already int32 (dense)."""
    from concourse import bacc, mybir, tile

    Alu = mybir.AluOpType
    Act = mybir.ActivationFunctionType
    f32 = mybir.dt.float32
    i32 = mybir.dt.int32

    lab_cols = 2 * L if pairs else L
    nc = bacc.Bacc("TRN2", target_bir_lowering=False, debug=False)
    out_d = nc.dram_tensor("out_f", [BC, 2 * L], f32, kind="ExternalInput")
    lab_d = nc.dram_tensor("lab_i", [BC, lab_cols], i32, kind="ExternalInput")
    acc_d = nc.dram_tensor("acc_out", [P, 3 * NT], f32, kind="ExternalOutput")

    with tile.TileContext(nc) as tc:
        with tc.tile_pool(name="io", bufs=3) as io, \
             tc.tile_pool(name="sc", bufs=2) as sc, \
             tc.tile_pool(name="accp", bufs=1) as accp:
            acc_v = accp.tile([P, 2 * NT], f32)   # DVE accum slots
            acc_s = accp.tile([P, NT], f32)       # ACT accum slots
            for t in range(NT):
                g = io.tile([P, 2 * L], f32, tag="g")
                lb = io.tile([P, lab_cols], i32, tag="lb")
                nc.sync.dma_start(out=g, in_=out_d.ap()[t * P:(t + 1) * P, :])
                nc.sync.dma_start(out=lb, in_=lab_d.ap()[t * P:(t + 1) * P, :])
                gv = g.rearrange("p (j c) -> p j c", c=2)
                o0 = gv[:, :, 0]
                o1 = gv[:, :, 1]
                if pairs:
                    m = lb.rearrange("p (j c) -> p j c", c=2)[:, :, 0]
                else:
                    m = lb[:, :]
                s0 = sc.tile([P, L], f32, tag="s0")
                s1 = sc.tile([P, L], f32, tag="s1")
                s2 = sc.tile([P, L], f32, tag="s2")
                nc.vector.scalar_tensor_tensor(
                    out=s0, in0=o0, scalar=W_POS, in1=m,
                    op0=Alu.mult, op1=Alu.mult,
                    accum_out=acc_v[:, 2 * t:2 * t + 1],
                )
                nc.vector.scalar_tensor_tensor(
                    out=s1, in0=o1, scalar=-W_NEG, in1=m,
                    op0=Alu.mult, op1=Alu.mult,
                    accum_out=acc_v[:, 2 * t + 1:2 * t + 2],
                )
                nc.scalar.activation(
                    out=s2, in_=o1, func=Act.Copy, scale=W_NEG,
                    accum_out=acc_s[:, t:t + 1],
                )
            nc.sync.dma_start(out=acc_d.ap()[:, 0:2 * NT], in_=acc_v)
            nc.sync.dma_start(out=acc_d.ap()[:, 2 * NT:3 * NT], in_=acc_s)
    nc.finalize()
    return nc


def _get_nc(pairs):
    if pairs not in _NC:
        _NC[pairs] = _build(pairs)
    return _NC[pairs]


def _ensure_ntff_hook():
    """The image's antenv package lacks axon_hooks; synthesize it and wire
    the ctypes NTFF-profiling hook so run_bass_kernel_spmd(trace=True)
    can capture HW exec times under axon."""
    import types

    try:
        import antenv.axon_hooks  # noqa: F401
        return
    except ImportError:
        pass
    import antenv

    mod = types.ModuleType("antenv.axon_hooks")
    mod._hook = None
    mod.set_axon_ntff_profile_hook = lambda h: setattr(mod, "_hook", h)
    mod.get_axon_ntff_profile_hook = lambda: mod._hook
    sys.modules["antenv.axon_hooks"] = mod
    antenv.axon_hooks = mod
    try:
        from trn_agent_boot.trn_boot import _ntff_profile_via_ctypes

        mod._hook = _ntff_profile_via_ctypes("/opt/axon/libaxon_pjrt.so")
    except Exception:
        pass


def _run(in_maps, pairs, trace=False):
    global LAST
    from concourse import bass_utils

    if trace:
        _ensure_ntff_hook()
        # artifact upload needs external storage; keep artifacts local
        bass_utils.upload_artifacts = lambda tmpdir: tmpdir

    LAST = bass_utils.run_bass_kernel_spmd(
        _get_nc(pairs), in_maps, core_ids=list(range(M)), trace=trace
    )
    return LAST


def kernel(output, labels):
    output = np.asarray(output)
    labels = np.asarray(labels)
    assert output.shape == (B, L, 2), output.shape
    assert labels.shape == (B, L), labels.shape
    out_f = np.ascontiguousarray(output).astype(np.float32, copy=False)
    out_f = out_f.reshape(B, 2 * L)
    if labels.dtype == np.int64:
        # int64 -> int32 pairs; little-endian, so even words hold the value
        pairs = True
        lab_i = np.ascontiguousarray(labels).view(np.int32).reshape(B, 2 * L)
    else:
        pairs = False
        lab_i = np.ascontiguousarray(labels).astype(np.int32, copy=False)
        lab_i = lab_i.reshape(B, L)

    in_maps = [
        {
            "out_f": out_f[k * BC:(k + 1) * BC],
            "lab_i": lab_i[k * BC:(k + 1) * BC],
        }
        for k in range(M)
    ]
    trace = bool(int(os.environ.get("BICUT_TRACE", "0")))
    res = _run(in_maps, pairs, trace=trace)
    total = 0.0
    for r in res.results:
        total += r["acc_out"].sum(dtype=np.float64)
    return np.array(total / B, dtype=np.float32)
